# revision 6
# baseline (speedup 1.0000x reference)
"""Betti3D loss kernel for Trainium2 (8 NeuronCores, data-parallel over batch).

Reference computation (see problem):
    p_down  = trilinear_resize(p_hat, (32, 32, 8))   # [B, C, 32, 32, 8]
    conf[b] = max(p_down[b, struct_id])
    out     = sum((1 - conf) * betti_error) / B

With input [B, C, 160, 160, 64] -> (32, 32, 8) the resize scales are exactly
(5, 5, 8), so with torch/jax half-pixel centers the source coordinates are:
    D axis: 5*i + 2      (weight exactly 0 -> pure gather)
    H axis: 5*j + 2      (weight exactly 0 -> pure gather)
    W axis: 8*k + 3.5    (weight exactly 0.5 -> 0.5*(x[8k+3] + x[8k+4]))
Therefore
    p_down[b, c, i, j, k] = 0.5 * (x[b,c,5i+2,5j+2,8k+3] + x[b,c,5i+2,5j+2,8k+4])
and conf[b] = 0.5 * max_{i,j,k} (x[...,8k+3] + x[...,8k+4]).  Since scaling by
0.5 commutes with max (and is exact in fp32), the device kernel computes
max(a+b) and the host multiplies by 0.5, reproducing the reference bit-exactly.

Per-core kernel (one batch sample per core):
  - one strided DMA gathers the 32x32 needed rows of 64 floats (256 KB) of
    channel struct_id into SBUF [128, 512]
  - DVE: add of the two W-phases (x[...,3::8] + x[...,4::8]) -> [128, 8, 8]
  - DVE: max-reduce over free dim -> [128, 1]
  - DMA out 128 partition maxima; host finishes max/mean over 8*128 values.

betti_error is 1 only for struct_id == 2 ('Myo'); for the other structures the
loss is exactly 0 and no device work is needed.
"""

import os

import numpy as np

_TARGETS = ((1, 0, 0), (1, 0, 0), (1, 1, 0), (1, 0, 0))
_BETTI_FALLBACK = (1, 0, 0)

_N_CORES = 8
_IN_SHAPE = (4, 160, 160, 64)  # per-sample [C, D, H, W]

_module_cache: dict = {}
LAST_RESULTS = None  # BassKernelResults of the most recent device run


def _patch_tail_drain():
    """Replace Tile's kernel-tail drain + all-engine barrier + semaphore
    cleanup with nothing.

    Two reasons.  (1) walrus in this image rejects >1 sem wait on any
    instruction ("Too many sync wait commands", CoreV3GenImpl
    setupSyncWait), which kills the stock multi-wait tail drain.  (2) The
    tail serializes on DMA-completion semaphores that post as 16 paced
    increments (~1.6 us for a short DMA) plus per-engine DRAIN ops that
    wait for the engine's DMA queues — about 2.5 us of pure epilogue for
    this kernel.  The NEFF-end runtime quiescence already guarantees all
    queues drain before outputs are read, and the runtime's per-run
    preamble re-initializes all semaphores, so for a single-shot kernel
    the Tile tail is redundant (verified bit-exact over 20+ HW reps)."""
    import concourse.tile as tile

    if getattr(tile.TileContext, "_betti_drain_patch", False):
        return

    def _drain_and_barrier(self, tick_clock, wait_clock):
        assert self.sems is not None
        popped = self.nc._tile_sem_poison_stack.pop()
        assert popped is self._sem_poison

    tile.TileContext._drain_and_barrier = _drain_and_barrier
    tile.TileContext._betti_drain_patch = True


def _ensure_ntff_hook():
    """Make trace=True safe anywhere: the image's antenv package lacks
    axon_hooks, whose absence crashes run_bass_kernel_spmd's trace path.
    Install a shim module and register the ctypes NTFF hook when available
    (hook=None degrades to bass_utils' graceful 'skip trace' path)."""
    import sys
    import types

    if "antenv.axon_hooks" not in sys.modules:
        try:
            import antenv.axon_hooks  # noqa: F401
        except ImportError:
            mod = types.ModuleType("antenv.axon_hooks")
            mod._hook = None
            mod.set_axon_ntff_profile_hook = lambda h: setattr(mod, "_hook", h)
            mod.get_axon_ntff_profile_hook = lambda: mod._hook
            sys.modules["antenv.axon_hooks"] = mod
            try:
                from trn_agent_boot.trn_boot import _ntff_profile_via_ctypes

                hook = _ntff_profile_via_ctypes("/opt/axon/libaxon_pjrt.so")
                if hook is not None:
                    mod.set_axon_ntff_profile_hook(hook)
            except Exception:
                pass
    # No S3 in this container; keep NTFF artifacts local.
    from concourse import bass_utils

    if getattr(bass_utils.upload_artifacts, "__name__", "") != "<lambda>":
        bass_utils.upload_artifacts = lambda tmpdir: tmpdir


def _strip_const_memsets(m):
    """Drop Bass.__init__ overhead this kernel doesn't need: the const-*
    memsets (they'd open the NTFF 'useful' window ~0.7 us early), the
    init all-engine barrier (Drain/EventSemaphore pairs — walrus's own
    starting CoreBarrier already aligns the engines), and register setup
    on the three engines (PE/Pool/ACT) that execute nothing."""
    idle = {"Pool", "Activation", "PE"}
    for function in m.functions:
        for block in function.blocks:
            keep = []
            for inst in block.instructions:
                tn = type(inst).__name__
                eng = str(getattr(inst, "engine", "")).split(".")[-1]
                if tn in ("InstDrain", "InstEventSemaphore"):
                    continue
                if tn == "InstMemset" and inst.outs and getattr(
                        inst.outs[0], "memref", "").startswith("const-"):
                    continue
                if eng in idle and tn in ("InstRegisterMove", "InstNoOp"):
                    continue
                keep.append(inst)
            if len(keep) != len(block.instructions):
                block.instructions[:] = keep


def _merge_blocks(m):
    """This kernel has no control flow: the main/tile/end basic blocks are
    chained by per-engine unconditional branches.  Fold everything into one
    block and drop the chaining branches (10 instructions + an IRAM block
    boundary on the critical Sync stream)."""
    for fn in m.functions:
        blocks = list(fn.blocks)
        if len(blocks) <= 1:
            continue
        names = [b.name for b in blocks]
        merged = []
        for bi, b in enumerate(blocks):
            nxt = names[bi + 1] if bi + 1 < len(names) else None
            for inst in b.instructions:
                if (type(inst).__name__ == "InstUnconditionalBranch"
                        and getattr(inst, "target", None) == nxt):
                    continue
                merged.append(inst)
        blocks[0].instructions[:] = merged
        fn.blocks[:] = [blocks[0]]


def _build(struct_id: int):
    import concourse.bass as bass
    import concourse.tile as tile
    from concourse import mybir

    _patch_tail_drain()

    nc = bass.Bass("TRN2", target_bir_lowering=False, debug=False,
                   num_devices=_N_CORES)
    x = nc.dram_tensor("x", list(_IN_SHAPE), mybir.dt.float32,
                       kind="ExternalInput").ap()
    o = nc.dram_tensor("o", [1], mybir.dt.float32,
                       kind="ExternalOutput").ap()
    with tile.TileContext(nc) as tc:
        with tc.tile_pool(name="p", bufs=1) as pool:
            # One strided DMA gathers the 1024 needed 256 B rows into a
            # 128-partition layout (p = i*4 + j//8, free = (j%8, w)).
            # A single producer keeps every consumer at one semaphore
            # wait (this toolchain rejects instructions with >1 wait).
            sub = x[struct_id, 2::5, 2::5, :]          # [32, 32, 64] strided
            t = pool.tile([128, 512], mybir.dt.float32)
            nc.sync.dma_start(t[:], sub)
            tv = t[:].rearrange("p (j w) -> p j w", w=64)
            scr = pool.tile([128, 64], mybir.dt.float32)
            sv = scr[:].rearrange("p (j k) -> p j k", k=8)
            # The measured exec window is [first compute-op start .. NEFF
            # quiescence], and quiescence waits on the output DMA's
            # completion semaphore.  128 per-partition 4 B writes into one
            # 512 B HBM region are sub-cacheline RMWs whose write receipts
            # trickle in for ~8 us.  So: reduce all the way to one scalar
            # on-chip and emit a single-descriptor output DMA instead.
            a = pool.tile([128, 32], mybir.dt.float32)
            nc.vector.tensor_tensor(out=sv, in0=tv[:, :, 3::8],
                                    in1=tv[:, :, 4::8],
                                    op=mybir.AluOpType.add)
            nc.vector.tensor_reduce(a[:, 0:1], scr[:],
                                    axis=mybir.AxisListType.X,
                                    op=mybir.AluOpType.max)
            # Four 32x32 block transposes: block q's column 0 (the maxima of
            # partitions 32q..32q+31) lands in partition 0, cols 32q..32q+31
            # of b.  Rows 1-31 carry don't-care bits (transpose only moves
            # bits, never computes on them).
            b = pool.tile([32, 128], mybir.dt.float32)
            for q in range(4):
                nc.vector.transpose(b[0:32, 32 * q:32 * (q + 1)],
                                    a[32 * q:32 * (q + 1), 0:32])
            c = pool.tile([1, 1], mybir.dt.float32)
            nc.vector.tensor_reduce(c[0:1, 0:1], b[0:1, 0:128],
                                    axis=mybir.AxisListType.X,
                                    op=mybir.AluOpType.max)
            # Single 4 B descriptor: one HBM write + one receipt, so the
            # NEFF-end quiescence wait collapses.
            nc.sync.dma_start(o[:], c[0:1, 0:1])
    _strip_const_memsets(nc.m)
    _merge_blocks(nc.m)
    return nc


def kernel(p_hat: np.ndarray, struct_id) -> np.ndarray:
    global LAST_RESULTS
    sid = int(struct_id)
    target = _TARGETS[sid]
    betti_error = sum(abs(_BETTI_FALLBACK[k] - target[k]) for k in range(3))
    B = p_hat.shape[0]
    if betti_error == 0:
        return np.zeros((), dtype=p_hat.dtype)

    from concourse import bass_utils

    assert B == _N_CORES and tuple(p_hat.shape[1:]) == _IN_SHAPE, (
        f"kernel hardcoded for shape (8, 4, 160, 160, 64), got {p_hat.shape}"
    )
    if sid not in _module_cache:
        _module_cache[sid] = _build(sid)
    nc = _module_cache[sid]

    p_hat = np.ascontiguousarray(p_hat, dtype=np.float32)
    in_maps = [{"x": p_hat[b]} for b in range(B)]
    trace = bool(int(os.environ.get("BETTI_TRACE", "0")))
    if trace or os.environ.get("BASS_TRACE"):
        _ensure_ntff_hook()
    res = bass_utils.run_bass_kernel_spmd(
        nc, in_maps, core_ids=list(range(_N_CORES)), trace=trace
    )
    LAST_RESULTS = res

    m = np.stack([r["o"].reshape(-1)[0] for r in res.results]).astype(
        np.float32)                                           # [8] max of (a+b)
    conf = np.float32(0.5) * m                                # exact scaling
    total = np.sum((np.float32(1.0) - conf) * np.float32(betti_error),
                   dtype=np.float32)
    out = total / np.float32(max(B, 1))
    return np.asarray(out, dtype=p_hat.dtype)

